# revision 83
# baseline (speedup 1.0000x reference)
"""Single-head attention (B=4, Lq=Lkv=4096, D=128) on 8 TRN2 NeuronCores.

Sharding: data-parallel over (batch, query-half). Core c handles batch c//2,
query rows (c%2)*2048 ... +2048, with full K/V for that batch. No collectives.

Per-core kernel, designed so the Scalar engine (ACT) does NOTHING but the 64
softmax-exp instructions (the roofline for this shape):

  - Softmax shift-invariance folds both projection biases out of the matmul
    path: softmax_k[(x1 Wq^T + bq)(x2 Wk^T + bk)^T] drops the per-q constant
    term exactly, leaving scores x1 G x2^T + t[k] with G = Wq^T Wk and
    t[k] = x2[k] (Wk^T bq). G is precomputed on the host; t is folded into
    the exp instruction's per-partition bias AP. The Q path therefore needs
    NO projection at all (qt = x1^T raw), and K needs a single G-projection.
  - Inputs are pre-cast to fp16 on the host, so the DMA XBAR transposes them
    straight from HBM into SBUF [d, k]-tiles: no PE transposes, no staging
    copies, no casts on any engine. Phase 1 uses only 2 PSUM banks (512-col
    projection chunks), so the main-loop PSUM pools (stp 4 + ot 2) never
    alias it and the main loop starts ~4us in.
  - Main loop per (k-tile, q-half): S^T = ktG_tile.T @ x1^T (fp16, full PE
    rate) into PSUM; ACT computes exp (scale + per-k bias folded) -> E^T fp16
    in SBUF; DVE accumulates E^T partial sums (fp16 2x mode, pair+chain); PE
    accumulates O^T += V_tile.T @ E^T in a single shared PSUM accumulator.
    Query halves run serially so epilogue(0) hides under half 1's main loop.
  - Epilogue per half: sumexp via all-ones matmuls accumulating the esum
    chain plus the last two E^T tiles directly in PSUM (PE is idle at the
    tail; DVE is its serial resource), reciprocal + multiply on DVE, chunked
    DMA out of O^T in fp16 (the host upcasts; halves the final transfer). Half 0's epilogue is deferred into half 1's loop (it
    must precede half 1's start=True O-accum); half 1 pre-emits its se
    matmuls so only the et31 matmuls, recips and muls trail the last exp.

Scheduling notes baked into the emission order (the hardware queues are
in-order and Tile's sem waits are cumulative per-engine ticks):
  - every DMA runs on the SP queue and is a DmaTransposeAnt (the const pack
    is loaded via a transpose too): mixed DMA types or queues chain serially
    on the shared DGE (~2.3us per transition);
  - projections are interleaved into the early iterations at their data-
    arrival points, O-accumulation trails S/exp by a few iterations (deeper
    in half 1, whose first o_acc waits epilogue-0's muls).

Numerics: x and K/V/Q operands in fp16 (f32 PSUM accumulation), softmax
without max-subtraction (|scores/sqrt(d)| < ~8, exp safe in fp32/fp16).
End-to-end scale-relative absmax error vs the fp32 reference: ~1e-3.
"""

import os
import sys

# Recovers wedged NeuronCores (NRT_EXEC_UNIT_UNRECOVERABLE) at init; must be
# set before the first device use.
os.environ.setdefault("NEURON_RT_RESET_CORES", "1")

if "/opt/trn_rl_repo" not in sys.path:
    sys.path.insert(0, "/opt/trn_rl_repo")

from contextlib import ExitStack

import numpy as np

import concourse.bass as bass  # noqa: F401  (bass types used via bacc/tile)
import concourse.bacc as bacc
import concourse.tile as tile
from concourse import mybir
from concourse._compat import with_exitstack
from concourse.bass_utils import run_bass_kernel_spmd

D = 128
LQ = 2048  # per-core query slab
LKV = 4096
QH = 1024  # query half processed per pass
NKT = LKV // 128  # 32 k-tiles
SCALE = float(1.0 / np.sqrt(128.0))

# Row-group splits for the DRAM->SBUF XBAR transposes (small first groups so
# the main loop starts early). x2/x3 splits match so k-tiles line up.
KV_SPLIT = [256, 256, 512, 1024, 2048]
Q_SPLIT = [1024, 1024]  # one group per query half

F32 = mybir.dt.float32
FP16 = mybir.dt.float16


@with_exitstack
def attn_body(ctx: ExitStack, tc: tile.TileContext, io: dict):
    nc = tc.nc
    ctx.enter_context(
        nc.allow_low_precision(reason="fp16 operands, fp32 PSUM accumulation")
    )
    out = io["o"]

    # ALL DMAs ride the single SP queue: Tile serializes DMA instructions
    # across different queues (cross-queue chaining on the shared DGE), while
    # consecutive same-queue DMAs pipeline freely.
    # All constants arrive in ONE XBAR-transpose load (same DMA type as the
    # input transposes, so nothing on the SP queue ever chains):
    # cp columns 0:128 = G^T [d,d'] | 128:256 = Wv^T [d,e] | 256:288 = per-
    # (k-tile,lane) exp bias | 288 = bv.
    consts = ctx.enter_context(tc.tile_pool(name="consts", bufs=1))
    cp = consts.tile([128, 432], FP16)
    GT = cp[:, 0:128]
    WvT = cp[:, 128:256]
    tb = cp[:, 256 : 256 + NKT]
    bv = consts.tile([128, 1], F32)  # tensor_scalar wants an f32 scalar AP
    ones_mat = consts.tile([128, 128], FP16)
    nc.gpsimd.memset(ones_mat, 1.0)

    # Persistent activations.
    acts = ctx.enter_context(tc.tile_pool(name="acts", bufs=1))
    qt_g = [
        acts.tile([128, R // 128, 128], FP16, tag=f"qt{g}", name=f"qt{g}")
        for g, R in enumerate(Q_SPLIT)
    ]
    x2t_g = [
        acts.tile([128, R // 128, 128], FP16, tag=f"x2t{g}", name=f"x2t{g}")
        for g, R in enumerate(KV_SPLIT)
    ]
    x3t_g = [
        acts.tile([128, R // 128, 128], FP16, tag=f"x3t{g}", name=f"x3t{g}")
        for g, R in enumerate(KV_SPLIT)
    ]
    ktg_g = [
        acts.tile([128, R], FP16, tag=f"ktg{g}", name=f"ktg{g}")
        for g, R in enumerate(KV_SPLIT)
    ]
    vn_g = [
        acts.tile([128, R // 128, 128], FP16, tag=f"vn{g}", name=f"vn{g}")
        for g, R in enumerate(KV_SPLIT)
    ]
    vt_g = [
        acts.tile([128, R], FP16, tag=f"vt{g}", name=f"vt{g}")
        for g, R in enumerate(KV_SPLIT)
    ]
    otn_h = [acts.tile([128, QH], FP16, tag=f"otn{i}", name=f"otn{i}") for i in range(2)]

    # (group, j) lookup for global k-tile / q-chunk indices.
    kv_tiles = []  # kt -> (g, j)
    for g, R in enumerate(KV_SPLIT):
        for j in range(R // 128):
            kv_tiles.append((g, j))
    def kt_tile(kt):
        g, j = kv_tiles[kt]
        return ktg_g[g][:, j * 128 : (j + 1) * 128]

    def vn_tile(kt):
        g, j = kv_tiles[kt]
        return vn_g[g][:, j, :]

    def qt_chunk(h, c):
        return qt_g[h][:, c * 4 : c * 4 + 4, :]

    pmm = ctx.enter_context(tc.tile_pool(name="pmm", bufs=2, space="PSUM"))
    stp = ctx.enter_context(tc.tile_pool(name="stp", bufs=2, space="PSUM"))
    otp = ctx.enter_context(tc.tile_pool(name="otp", bufs=1, space="PSUM"))
    etp = ctx.enter_context(tc.tile_pool(name="etp", bufs=14))
    sumt = ctx.enter_context(tc.tile_pool(name="sumt", bufs=6))
    nrm = ctx.enter_context(tc.tile_pool(name="nrm", bufs=2))

    def xpose(xin, dst, g, split):
        """XBAR transpose straight from HBM (fp16): x[off:off+R, :] ->
        [d, j, k%128] tiles in SBUF. Natural k order, no staging."""
        R = split[g]
        off = sum(split[:g])
        nc.sync.dma_start_transpose(out=dst, in_=xin[off : off + R, :])

    def kproj_chunk(g, c0, on_act=False):
        R = KV_SPLIT[g]
        w = min(512, R - c0)
        ps = pmm.tile([128, 512], F32, tag="pj")
        nc.tensor.matmul(
            ps[:, 0:w],
            GT,
            x2t_g[g][:, c0 // 128 : (c0 + w) // 128, :],
            start=True,
            stop=True,
        )
        if on_act:
            nc.scalar.copy(out=ktg_g[g][:, c0 : c0 + w], in_=ps[:, 0:w])
        else:
            nc.vector.tensor_copy(out=ktg_g[g][:, c0 : c0 + w], in_=ps[:, 0:w])

    def kproj(g, on_act=False):
        """ktG group: 512-col chunks through 1-bank PSUM tiles. The earliest
        groups' PSUM->SBUF moves ride ACT (idle until the first exp); later
        groups use DVE."""
        for c0 in range(0, KV_SPLIT[g], 512):
            kproj_chunk(g, c0, on_act)

    def vproj_chunk(g, c0):
        R = KV_SPLIT[g]
        w = min(512, R - c0)
        ps = pmm.tile([128, 512], F32, tag="pj")
        nc.tensor.matmul(
            ps[:, 0:w],
            WvT,
            x3t_g[g][:, c0 // 128 : (c0 + w) // 128, :],
            start=True,
            stop=True,
        )
        nc.vector.tensor_scalar_add(
            out=vt_g[g][:, c0 : c0 + w], in0=ps[:, 0:w], scalar1=bv
        )

    def vproj(g):
        for c0 in range(0, KV_SPLIT[g], 512):
            vproj_chunk(g, c0)

    # ---- Phase 1 emission. The shared DGE serializes across DMA *type*
    # transitions (copy<->transpose) but pipelines same-type runs, so the SP
    # queue carries ONLY transposes (the const pack included); the output
    # copies follow at the very end where the one type-switch is free.
    nc.sync.dma_start_transpose(out=cp, in_=io["cpackT"])
    nc.vector.tensor_copy(out=bv, in_=cp[:, 288:289])
    xpose(io["x2"], x2t_g[0], 0, KV_SPLIT)
    xpose(io["x1"], qt_g[0], 0, Q_SPLIT)
    xpose(io["x2"], x2t_g[1], 1, KV_SPLIT)
    xpose(io["x3"], x3t_g[0], 0, KV_SPLIT)
    xpose(io["x2"], x2t_g[2], 2, KV_SPLIT)
    nc.sync.dma_start_transpose(out=vn_g[0], in_=vt_g[0])
    xpose(io["x3"], x3t_g[1], 1, KV_SPLIT)
    xpose(io["x2"], x2t_g[3], 3, KV_SPLIT)
    nc.sync.dma_start_transpose(out=vn_g[1], in_=vt_g[1])
    xpose(io["x3"], x3t_g[2], 2, KV_SPLIT)
    xpose(io["x2"], x2t_g[4], 4, KV_SPLIT)
    xpose(io["x3"], x3t_g[3], 3, KV_SPLIT)
    nc.sync.dma_start_transpose(out=vn_g[2], in_=vt_g[2])
    xpose(io["x1"], qt_g[1], 1, Q_SPLIT)
    xpose(io["x3"], x3t_g[4], 4, KV_SPLIT)
    nc.sync.dma_start_transpose(out=vn_g[3], in_=vt_g[3])
    nc.sync.dma_start_transpose(out=vn_g[4], in_=vt_g[4])

    # ---- Phase 2: attention main loop, query halves serial ----
    # E^T partial sums kept per-chunk: 2x512 for half 0 (its epilogue hides
    # under half 1's main loop), 4x256 for half 1 (shortens the exposed tail
    # chain: sums -> se -> recip -> mul -> out).
    ot = otp.tile([128, QH], F32, tag="ot", name="ot")
    NCH = [2, 2]
    pendings = [dict(), dict()]
    chains = [[None] * NCH[0], [None] * NCH[1]]

    last_ets = [[], []]  # [h] -> final two E^T tiles, summed by the se matmul

    def sum_insert(h, tile_, kt):
        if kt >= NKT - 2:
            # The last two tiles skip the DVE chain: the epilogue's ones-
            # matmul accumulates them into se directly (PE is idle there,
            # DVE is the tail's serial resource).
            last_ets[h].append(tile_)
            return
        pending = pendings[h]
        if 0 not in pending:
            pending[0] = tile_
            return
        prev = pending.pop(0)
        w = QH // NCH[h]
        for c in range(NCH[h]):
            sl = slice(c * w, (c + 1) * w)
            pair = sumt.tile([128, w], FP16, tag=f"sum0_{h}_{c}", name="s0")
            nc.vector.tensor_add(out=pair, in0=prev[:, sl], in1=tile_[:, sl])
            if chains[h][c] is None:
                chains[h][c] = pair
            else:
                acc = sumt.tile([128, w], FP16, tag=f"sumc_{h}_{c}", name="sc")
                nc.vector.tensor_add(out=acc, in0=chains[h][c], in1=pair)
                chains[h][c] = acc

    ets = [[None] * NKT, [None] * NKT]

    def s_exp(kt, h, split_exp=False):
        st = stp.tile([128, QH], F32, tag="st", name="st")
        for c in range(2):
            sl = slice(c * 512, (c + 1) * 512)
            nc.tensor.matmul(st[:, sl], kt_tile(kt), qt_chunk(h, c), start=True, stop=True)
        et = etp.tile([128, QH], FP16, tag="et", name="et")
        if split_exp:
            # Last iteration: 2x512 exps so the tail's sum chain starts a
            # half-exp earlier (costs one extra ACT instruction overhead).
            for c in range(2):
                sl = slice(c * 512, (c + 1) * 512)
                nc.scalar.activation(
                    out=et[:, sl],
                    in_=st[:, sl],
                    func=mybir.ActivationFunctionType.Exp,
                    scale=SCALE,
                    bias=tb[:, kt : kt + 1],
                )
        else:
            nc.scalar.activation(
                out=et,
                in_=st,
                func=mybir.ActivationFunctionType.Exp,
                scale=SCALE,
                bias=tb[:, kt : kt + 1],
            )
        ets[h][kt] = et
        sum_insert(h, et, kt)

    def o_acc(kt, h, chunks=(0, 1)):
        for c in chunks:
            sl = slice(c * 512, (c + 1) * 512)
            nc.tensor.matmul(
                ot[:, sl], vn_tile(kt), ets[h][kt][:, sl],
                start=kt == 0, stop=kt == NKT - 1,
            )

    se_pre = [None, None]

    def epi_pre(h):
        # chain+et30 ones-matmuls: inputs exist by exp(NKT-2), so these run
        # under the final iterations, leaving only the et31 matmuls after
        # the last exp.
        w = QH // NCH[h]
        se_tiles = [pmm.tile([128, 512], F32, tag="pj", name="se") for _ in range(NCH[h])]
        for c in range(NCH[h]):
            sl = slice(c * w, (c + 1) * w)
            nc.tensor.matmul(
                se_tiles[c][:, 0:w], ones_mat, chains[h][c], start=True, stop=False
            )
            nc.tensor.matmul(
                se_tiles[c][:, 0:w], ones_mat, last_ets[h][0][:, sl], start=False, stop=False
            )
        se_pre[h] = se_tiles

    def epilogue_tail(h):
        # PE order: et31 se-matmuls first (recips chase them on DVE), THEN
        # the final O-writes -- ot WAR tracking is whole-tile, so the muls
        # must come after ALL O-accumulation regardless.
        q0 = h * QH
        w = QH // NCH[h]
        recs = []
        for c in range(NCH[h]):
            sl = slice(c * w, (c + 1) * w)
            nc.tensor.matmul(
                se_pre[h][c][:, 0:w], ones_mat, last_ets[h][1][:, sl], start=False, stop=True
            )
            rec = nrm.tile([128, w], F32, tag=f"rec{h}", name="rec")
            nc.vector.reciprocal(out=rec, in_=se_pre[h][c][:, 0:w])
            recs.append(rec)
        o_acc(NKT - 2, h)
        o_acc(NKT - 1, h)
        for c in range(NCH[h]):
            sl = slice(c * w, (c + 1) * w)
            nc.vector.tensor_mul(out=otn_h[h][:, sl], in0=ot[:, sl], in1=recs[c])
            nc.sync.dma_start(
                out=out[:, q0 + c * w : q0 + (c + 1) * w], in_=otn_h[h][:, sl]
            )

    def epilogue(h):
        q0 = h * QH
        # Partition-reduce each esum chunk with an all-ones stationary so the
        # result lands replicated across partitions; recip/mul/DMA chunked so
        # the chain pipelines. se borrows an st slot; the normalize multiply
        # reads the O^T PSUM accumulator directly.
        w = QH // NCH[h]
        # se lives in the (long-retired) pmm banks, NOT an st slot --
        # borrowing stp here would stall the other half's st rotation for
        # the whole epilogue. The et31 matmuls come last (latest producer).
        se_tiles = [pmm.tile([128, 512], F32, tag="pj", name="se") for _ in range(NCH[h])]
        for c in range(NCH[h]):
            sl = slice(c * w, (c + 1) * w)
            nc.tensor.matmul(
                se_tiles[c][:, 0:w], ones_mat, chains[h][c], start=True, stop=False
            )
            nc.tensor.matmul(
                se_tiles[c][:, 0:w], ones_mat, last_ets[h][0][:, sl], start=False, stop=False
            )
        for c in range(NCH[h]):
            sl = slice(c * w, (c + 1) * w)
            nc.tensor.matmul(
                se_tiles[c][:, 0:w], ones_mat, last_ets[h][1][:, sl], start=False, stop=True
            )
        recs = []
        for c in range(NCH[h]):
            rec = nrm.tile([128, w], F32, tag=f"rec{h}", name="rec")
            nc.vector.reciprocal(out=rec, in_=se_tiles[c][:, 0:w])
            recs.append(rec)
        for c in range(NCH[h]):
            sl = slice(c * w, (c + 1) * w)
            nc.vector.tensor_mul(out=otn_h[h][:, sl], in0=ot[:, sl], in1=recs[c])
            nc.sync.dma_start(
                out=out[:, q0 + c * w : q0 + (c + 1) * w], in_=otn_h[h][:, sl]
            )

    # Projections are interleaved into the early iterations in data-arrival
    # order: exp_i's wait is a cumulative PE tick, so any PE matmul scheduled
    # before S_i transitively gates it -- a projection placed too early in
    # the PE stream (before its transpose lands) stalls every later exp.
    LAG = 2  # O-accumulation emitted LAG iterations behind S/exp

    def run_half(h, phase1=()):
        extras = dict(phase1)
        # H1 runs a deeper O-lag: its first o_acc waits epilogue(0)'s muls,
        # and anything queued behind it on the in-order PE stream would stall
        # -- so keep several S-matmuls ahead of it.
        lag = LAG if h == 0 else 8
        for kt in range(NKT):
            s_exp(kt, h, split_exp=(h == 1 and kt == NKT - 1))
            if kt in extras:
                extras[kt]()
            if h == 1 and kt == NKT - 2:
                epi_pre(1)
            if h == 1 and kt == lag:
                # Must precede H1's first o_acc: its start=True resets the
                # shared PSUM accumulator that epilogue(0) still reads.
                epilogue(0)
            if kt >= lag:
                o_acc(kt - lag, h)
        for kt in range(NKT - lag, NKT - 2 if h == 1 else NKT):
            o_acc(kt, h)

    kproj(0, on_act=True)
    kproj(1, on_act=True)
    run_half(0, phase1=[
        (0, lambda: (vproj(0), kproj(2))),
        (1, lambda: vproj(1)),
        (3, lambda: kproj_chunk(3, 0)),
        (4, lambda: kproj_chunk(3, 512)),
        (5, lambda: vproj(2)),
        (7, lambda: kproj_chunk(4, 0)),
        (8, lambda: kproj_chunk(4, 512)),
        (9, lambda: (kproj_chunk(4, 1024), vproj_chunk(3, 0))),
        (10, lambda: (kproj_chunk(4, 1536), vproj_chunk(3, 512))),
        (12, lambda: vproj_chunk(4, 0)),
        (13, lambda: vproj_chunk(4, 512)),
        (14, lambda: vproj_chunk(4, 1024)),
        (15, lambda: vproj_chunk(4, 1536)),
    ])
    run_half(1)
    epilogue_tail(1)


def build_nc() -> "bacc.Bacc":
    nc = bacc.Bacc("TRN2", target_bir_lowering=False, debug=False, num_devices=8)
    io = {}
    io["x1"] = nc.dram_tensor("x1", [LQ, D], FP16, kind="ExternalInput").ap()
    io["x2"] = nc.dram_tensor("x2", [LKV, D], FP16, kind="ExternalInput").ap()
    io["x3"] = nc.dram_tensor("x3", [LKV, D], FP16, kind="ExternalInput").ap()
    io["cpackT"] = nc.dram_tensor("cpackT", [432, D], FP16, kind="ExternalInput").ap()
    io["o"] = nc.dram_tensor("o", [128, LQ], FP16, kind="ExternalOutput").ap()
    with tile.TileContext(nc) as tc:
        attn_body(tc, io)
    nc.compile()
    return nc


def make_in_maps(inputs: dict) -> list[dict]:
    Wq = np.asarray(inputs["Wq"], np.float32)
    Wk = np.asarray(inputs["Wk"], np.float32)
    Wv = np.asarray(inputs["Wv"], np.float32)
    bq = np.asarray(inputs["bq"], np.float32)
    bv = np.asarray(inputs["bv"], np.float32)
    u = Wk.T @ bq  # t[k] = x2[k] . u  (bq.bk constant drops in softmax)
    x1 = np.asarray(inputs["x1"], np.float32)
    x2 = np.asarray(inputs["x2"], np.float32)
    x3 = np.asarray(inputs["x3"], np.float32)
    x1h = x1.astype(np.float16)
    x2h = x2.astype(np.float16)
    x3h = x3.astype(np.float16)
    in_maps = []
    for c in range(8):
        b, qh = c // 2, c % 2
        tvec = (x2[b] @ u) * SCALE  # [LKV]
        # cpackT rows: G^T^T=Wq^T Wk | Wv | tvec tiles | bv | pad. The device
        # transposes it back, so cp[:, j] = cpackT[j, :] per column j.
        cpackT = np.zeros((432, 128), np.float16)
        cpackT[0:128] = (Wq.T @ Wk).astype(np.float16)
        cpackT[128:256] = Wv.astype(np.float16)
        cpackT[256 : 256 + NKT] = tvec.reshape(NKT, 128).astype(np.float16)
        cpackT[288] = bv.astype(np.float16)
        in_maps.append(
            {
                "x1": np.ascontiguousarray(x1h[b, qh * LQ : (qh + 1) * LQ, :]),
                "x2": np.ascontiguousarray(x2h[b]),
                "x3": np.ascontiguousarray(x3h[b]),
                "cpackT": np.ascontiguousarray(cpackT),
            }
        )
    return in_maps


_NC_CACHE = None


def get_nc():
    global _NC_CACHE
    if _NC_CACHE is None:
        _NC_CACHE = build_nc()
    return _NC_CACHE


def kernel(**inputs) -> np.ndarray:
    nc = get_nc()
    in_maps = make_in_maps(inputs)
    res = run_bass_kernel_spmd(nc, in_maps, core_ids=list(range(8)))
    out = np.empty((4, 4096, 128), np.float32)
    for c in range(8):
        b, qh = c // 2, c % 2
        out[b, qh * LQ : (qh + 1) * LQ, :] = res.results[c]["o"].T.astype(np.float32)
    return out


if __name__ == "__main__":
    nc = build_nc()
    print("built OK")
